# revision 30
# baseline (speedup 1.0000x reference)
"""GPT-J joint attention (B=1, S=2048, D=2048, H=16, HD=128) on 8 Trainium2
NeuronCores, tensor-parallel over heads (2 heads per core).

Per-core program (all matmuls bf16 inputs, fp32 PSUM accumulation):
  - QT/KT = W[qk]_shard @ hidden^T        ([hd, s] layout, per head)
  - RoPE applied via a rotation-matrix matmul + elementwise combine
  - V = hidden @ Wv_shard^T               ([s, hd] layout)
  - scores^T tiles = KT_tile^T . QT_block ([k, q] layout) -> exp -> causal mask
  - O^T accumulated as V_tile^T . P^T; softmax denominator via ones-matmul
  - partial out = O^T{normalized}^T . Wo_shard^T, streamed to DRAM per row-block

Host side: shard/transpose/cast inputs, run SPMD on 8 cores, sum the 8
partial outputs (the tensor-parallel all-reduce equivalent).
"""
import sys

import numpy as np
import ml_dtypes

try:
    import concourse.bass as bass
except ImportError:  # pragma: no cover
    sys.path.insert(0, "/opt/trn_rl_repo")
    import concourse.bass as bass

import concourse.mybir as mybir
import concourse.tile as tile
from concourse.bass_utils import run_bass_kernel_spmd

BF16 = mybir.dt.bfloat16
F32 = mybir.dt.float32
NPBF16 = ml_dtypes.bfloat16

N_CORES = 8
S = 2048          # sequence length
D = 2048          # model dim
HD = 128          # head dim
NHC = 2           # heads per core
DC = NHC * HD     # shard width (256)
P = 128           # partitions
KD = D // P       # 16 contraction tiles over model dim
QBS = 512         # q-block size
NQB = S // QBS    # 4 q-blocks
NST = S // P      # 16 sequence tiles of 128
SCALE = 1.0 / float(np.sqrt(HD))

# ---------------------------------------------------------------------------
# Walrus's CoreV3 drain encoding accepts a single sem wait; Tile's tail drain
# carries one wait per logical proc. Split it into one drain per proc.
# ---------------------------------------------------------------------------


def _install_drain_split():
    if getattr(tile.TileContext, "_drain_split_installed", False):
        return
    from concourse.vector_clock import ScopedClock, VectorClock

    def _drain_and_barrier(self, tick_clock, wait_clock):
        full = tick_clock.global_clock
        n = len(full)
        for i in range(n):
            if full[i] <= 0:
                continue
            vec = [full[j] if j == i else 0 for j in range(n)]
            drain_inst = self.nc.sync.drain()
            wait_clock.add_sem_waits(
                drain_inst.ins, ScopedClock({None: VectorClock(vec)})
            )
        self.nc.all_engine_barrier()
        assert self.sems is not None
        popped = self.nc._tile_sem_poison_stack.pop()
        assert popped is self._sem_poison
        self.nc.clear_and_free_semaphores(list(self.sems.allocated().values()))
        self.nc.all_engine_barrier()

    tile.TileContext._drain_and_barrier = _drain_and_barrier
    tile.TileContext._drain_split_installed = True


def _split_excess_waits(nc, limit=1):
    """This walrus build rejects instructions carrying more than one sem wait
    (CoreV3 setupSyncWait: 'Too many sync wait commands'). Spill excess waits
    onto same-engine NOPs inserted just before the instruction — the engine
    executes them in queue order, so blocking semantics are unchanged."""
    ctr = 0
    for fn in nc.m.functions:
        for blk in fn.blocks:
            new_list = []
            for inst in blk.instructions:
                si = inst.sync_info
                if si is not None and len(si.on_wait) > limit:
                    waits = list(si.on_wait)
                    excess, keep = waits[:-limit], waits[-limit:]
                    for w in excess:
                        ctr += 1
                        nop = mybir.InstNoOp(
                            name=f"I-wsplit-{ctr}", text_hint="wait_split"
                        )
                        nop.engine = inst.engine
                        nop.sync_info = mybir.SyncInfo(on_wait=[w], on_update=[])
                        new_list.append(nop)
                    inst.sync_info = mybir.SyncInfo(
                        on_wait=keep, on_update=si.on_update
                    )
                new_list.append(inst)
            if len(new_list) != len(blk.instructions):
                blk.instructions[:] = new_list
    return ctr


def build_nc(split_waits=True):
    _install_drain_split()
    nc = bass.Bass()

    hT = nc.dram_tensor("hT", [D, S], BF16, kind="ExternalInput")
    wq = nc.dram_tensor("wq", [D, DC], BF16, kind="ExternalInput")
    wk = nc.dram_tensor("wk", [D, DC], BF16, kind="ExternalInput")
    wv = nc.dram_tensor("wv", [D, DC], BF16, kind="ExternalInput")
    wo = nc.dram_tensor("wo", [DC, D], BF16, kind="ExternalInput")
    ct = nc.dram_tensor("ct", [P, S], BF16, kind="ExternalInput")
    st = nc.dram_tensor("st", [P, S], BF16, kind="ExternalInput")
    rot = nc.dram_tensor("rot", [P, P], BF16, kind="ExternalInput")
    out = nc.dram_tensor("out", [S, D], BF16, kind="ExternalOutput")

    Exp = mybir.ActivationFunctionType.Exp
    Copy = mybir.ActivationFunctionType.Copy

    with tile.TileContext(nc) as tc:
        with (
            tc.tile_pool(name="const", bufs=1) as const,
            tc.tile_pool(name="acts", bufs=1) as acts,
            tc.tile_pool(name="work", bufs=2) as work,
            tc.tile_pool(name="ptpool", bufs=4) as ptpool,
            tc.tile_pool(name="outstage", bufs=2) as outstage,
            tc.tile_pool(name="ps_main", bufs=4, space="PSUM") as ps_main,
            tc.tile_pool(name="ps_acc", bufs=2, space="PSUM") as ps_acc,
            tc.tile_pool(name="ps_misc", bufs=1, space="PSUM") as ps_misc,
        ):
            # ---- constants / weights into SBUF (few big DMAs: the HWDGE
            # trigger sequencer costs ~0.4us per dma_start, so batch) ----
            wq_sb = const.tile([P, KD, DC], BF16)
            wk_sb = const.tile([P, KD, DC], BF16)
            wv_sb = const.tile([P, KD, DC], BF16)
            hT_sb = const.tile([P, KD, S], BF16)
            hT_r = hT.rearrange("(kd p) s -> p kd s", p=P)
            ct_sb = const.tile([P, S], BF16)
            st_sb = const.tile([P, S], BF16)
            rot_sb = const.tile([P, P], BF16)
            wo_sb = const.tile([P, NHC, D], BF16)
            # order = first-use order: Wq, then the first q-block of hidden
            # (in 4 kd-chunks so matmuls can start on partial data), then the
            # other weights / rope constants, then the rest of hidden.
            nc.sync.dma_start(
                out=wq_sb, in_=wq.rearrange("(kd p) e -> p kd e", p=P)
            )
            for c4 in range(4):
                nc.sync.dma_start(
                    out=hT_sb[:, c4 * 4:(c4 + 1) * 4, 0:QBS],
                    in_=hT_r[:, c4 * 4:(c4 + 1) * 4, 0:QBS],
                )
            nc.sync.dma_start(
                out=wk_sb, in_=wk.rearrange("(kd p) e -> p kd e", p=P)
            )
            nc.sync.dma_start(out=rot_sb, in_=rot[:, :])
            nc.sync.dma_start(out=ct_sb, in_=ct[:, :])
            nc.sync.dma_start(out=st_sb, in_=st[:, :])
            nc.sync.dma_start(
                out=wv_sb, in_=wv.rearrange("(kd p) e -> p kd e", p=P)
            )
            for qb in range(1, NQB):
                nc.sync.dma_start(
                    out=hT_sb[:, :, qb * QBS:(qb + 1) * QBS],
                    in_=hT_r[:, :, qb * QBS:(qb + 1) * QBS],
                )
            nc.sync.dma_start(
                out=wo_sb, in_=wo.rearrange("(k2 p) e -> p k2 e", p=P)
            )
            ones_colb = const.tile([P, 1], BF16)   # lhsT for k-partition sums
            nc.vector.memset(ones_colb, 1.0)
            ones_row = const.tile([1, P], F32)     # lhsT for partition broadcast
            nc.vector.memset(ones_row, 1.0)

            # persistent activations
            qt_sb = acts.tile([P, NHC, S], BF16)   # [hd, h, s] rotary-applied Q^T
            kt_sb = acts.tile([P, NHC, S], BF16)
            v_sb = acts.tile([P, NST, DC], BF16)   # [s%128, s//128, head*hd]
            otb_sb = acts.tile([P, NHC, S], BF16)  # normalized O^T per head

            # ---- projections, q-block at a time (follows hT DMA order) ----
            # The rope epilogue (psum->sbuf copy, rotation matmul, 3 DVE ops)
            # of each 16-matmul block is deferred until after the NEXT block's
            # matmuls are emitted, so the PE never waits on it.
            pending = []  # (psum, dst_sb, h, qb)

            def flush_rope(keep=0):
                while len(pending) > keep:
                    ps, dst_sb, h, qb = pending.pop(0)
                    sl = slice(qb * QBS, (qb + 1) * QBS)
                    raw = work.tile([P, QBS], BF16, tag="raw")
                    nc.scalar.activation(raw, ps, Copy)
                    rps = ps_main.tile([P, QBS], F32, tag="mm")
                    nc.tensor.matmul(rps, lhsT=rot_sb, rhs=raw, start=True, stop=True)
                    t1 = work.tile([P, QBS], BF16, tag="t1")
                    t2 = work.tile([P, QBS], BF16, tag="t2")
                    nc.vector.tensor_mul(t1, raw, ct_sb[:, sl])
                    nc.vector.tensor_mul(t2, rps, st_sb[:, sl])
                    nc.vector.tensor_add(dst_sb[:, h, sl], t1, t2)

            def project(w_sb, dst_sb, h, qb):
                flush_rope(keep=1)
                sl = slice(qb * QBS, (qb + 1) * QBS)
                ps = ps_main.tile([P, QBS], F32, name="proj_ps", tag="mm")
                for kd in range(KD):
                    nc.tensor.matmul(
                        ps,
                        lhsT=w_sb[:, kd, h * HD:(h + 1) * HD],
                        rhs=hT_sb[:, kd, sl],
                        start=(kd == 0),
                        stop=(kd == KD - 1),
                    )
                pending.append((ps, dst_sb, h, qb))

            for qb in range(NQB):
                for h in range(NHC):
                    project(wq_sb, qt_sb, h, qb)
                    project(wk_sb, kt_sb, h, qb)
                for s4 in range(4):
                    flush_rope(keep=1)
                    st_idx = qb * 4 + s4
                    ps = ps_main.tile([P, DC], F32, tag="mm")
                    for kd in range(KD):
                        nc.tensor.matmul(
                            ps,
                            lhsT=hT_sb[:, kd, st_idx * P:(st_idx + 1) * P],
                            rhs=wv_sb[:, kd, :],
                            start=(kd == 0),
                            stop=(kd == KD - 1),
                        )
                    nc.scalar.activation(v_sb[:, st_idx, :], ps, Copy)
            flush_rope()

            # ---- attention + out-projection, pipelined per q-block ----
            # Both heads interleave inside one kt loop (fills PE bubbles while
            # exp/mask run), the P.V matmul lags the score matmul by one kt,
            # and out_proj for block qb-1 runs during qb's denominator chain.
            def make_outproj_chunks(qb):
                """One chunk = one [128,512] out tile (2 matmuls + a copy,
                plus the row-block DMA on the last chunk). Chunks are fed one
                at a time into the next q-block's attention loop so the PE
                fills the bubbles left by the exp-paced score pipeline."""
                state = {}
                chunks = []
                for s4 in range(4):
                    st_idx = qb * 4 + s4
                    for eb in range(NQB):
                        def chunk(s4=s4, st_idx=st_idx, eb=eb):
                            if eb == 0:
                                state[s4] = outstage.tile(
                                    [P, D], BF16, name="ost", tag="ost"
                                )
                            ost = state[s4]
                            ops = ps_main.tile(
                                [P, QBS], F32, name="ops", tag="mm"
                            )
                            for h in range(NHC):
                                nc.tensor.matmul(
                                    ops,
                                    lhsT=otb_sb[:, h, st_idx * P:(st_idx + 1) * P],
                                    rhs=wo_sb[:, h, eb * QBS:(eb + 1) * QBS],
                                    start=(h == 0),
                                    stop=(h == NHC - 1),
                                )
                            nc.vector.tensor_copy(
                                ost[:, eb * QBS:(eb + 1) * QBS], ops
                            )
                            if eb == NQB - 1:
                                nc.sync.dma_start(
                                    out=out[st_idx * P:(st_idx + 1) * P, :],
                                    in_=ost,
                                )
                        chunks.append(chunk)
                return chunks

            OT_LAG = 2  # P.V matmul trails the score matmul by 2 kt steps so
            # its sem wait is already satisfied and LDWEIGHTS pipelines.
            pending_chunks = []
            for qb in range(NQB):
                qsl = slice(qb * QBS, (qb + 1) * QBS)
                kmax = (qb + 1) * 4
                ot_pss, pts = [], {}
                # 4-lane bf16 partial sums of exp tiles (softmax denominator);
                # all adds run in the DVE 16-bit fast mode, chains stay short.
                accs = [[None] * 4 for _ in range(NHC)]

                def acc_pt(h, kt, pt):
                    lane = kt % 4
                    if accs[h][lane] is None:
                        acc = work.tile(
                            [P, QBS], BF16, name=f"za{h}_{lane}",
                            tag=f"za{h}_{lane}",
                        )
                        nc.vector.tensor_copy(acc, pt)
                        accs[h][lane] = acc
                    else:
                        acc = accs[h][lane]
                        nc.vector.tensor_add(acc, acc, pt)

                def acc_merge(h):
                    lanes = [a for a in accs[h] if a is not None]
                    while len(lanes) > 1:
                        nxt = []
                        for i in range(0, len(lanes) - 1, 2):
                            nc.vector.tensor_add(lanes[i], lanes[i], lanes[i + 1])
                            nxt.append(lanes[i])
                        if len(lanes) % 2:
                            nxt.append(lanes[-1])
                        lanes = nxt
                    return lanes[0]

                for h in range(NHC):
                    ot_pss.append(ps_acc.tile([P, QBS], F32, name="ot_ps", tag="ps_ot"))

                def pv_step(kt):
                    for h in range(NHC):
                        nc.tensor.matmul(
                            ot_pss[h],
                            lhsT=v_sb[:, kt, h * HD:(h + 1) * HD],
                            rhs=pts[(h, kt)],
                            start=(kt == 0),
                            stop=(kt == kmax - 1),
                            skip_group_check=True,
                        )

                for kt in range(kmax):
                    for h in range(NHC):
                        sps = ps_main.tile([P, QBS], F32, tag="mm")
                        nc.tensor.matmul(
                            sps,
                            lhsT=kt_sb[:, h, kt * P:(kt + 1) * P],
                            rhs=qt_sb[:, h, qsl],
                            start=True,
                            stop=True,
                        )
                        pt = ptpool.tile([P, QBS], BF16, tag=f"pt{h}")
                        nc.scalar.activation(pt, sps, Exp, scale=SCALE)
                        j = kt - qb * 4
                        if j >= 0:  # diagonal tile: causal mask (on Pool);
                            # only columns < 128*(j+1) can be masked
                            w = min(P * (j + 1), QBS)
                            nc.gpsimd.affine_select(
                                out=pt[:, 0:w],
                                in_=pt[:, 0:w],
                                compare_op=mybir.AluOpType.is_ge,
                                fill=0.0,
                                base=qb * QBS - kt * P,
                                pattern=[[1, w]],
                                channel_multiplier=-1,
                            )
                        pts[(h, kt)] = pt
                        acc_pt(h, kt, pt)
                        if pending_chunks:
                            pending_chunks.pop(0)()
                    if kt >= OT_LAG:
                        pv_step(kt - OT_LAG)
                for kt in range(max(kmax - OT_LAG, 0), kmax):
                    pv_step(kt)

                den_sbs = []
                for h in range(NHC):
                    lanes = [a for a in accs[h] if a is not None]
                    den_ps = ps_misc.tile([1, QBS], F32, tag="ps_den")
                    for li, lane in enumerate(lanes):
                        nc.tensor.matmul(
                            den_ps, lhsT=ones_colb, rhs=lane,
                            start=(li == 0), stop=(li == len(lanes) - 1),
                            skip_group_check=True,
                        )
                    r_sb = work.tile([1, QBS], F32, tag=f"r{h}")
                    nc.vector.reciprocal(r_sb, den_ps)
                    den_sbs.append(r_sb)
                while pending_chunks:
                    pending_chunks.pop(0)()
                for h in range(NHC):
                    bc_ps = ps_misc.tile([P, QBS], F32, tag="ps_bc")
                    nc.tensor.matmul(
                        bc_ps, lhsT=ones_row, rhs=den_sbs[h], start=True, stop=True
                    )
                    bc_sb = work.tile([P, QBS], F32, tag=f"bc{h}")
                    nc.vector.tensor_copy(bc_sb, bc_ps)
                    nc.vector.tensor_mul(otb_sb[:, h, qsl], ot_pss[h], bc_sb)
                pending_chunks = make_outproj_chunks(qb)

            while pending_chunks:
                pending_chunks.pop(0)()
    if split_waits:
        _split_excess_waits(nc)
    return nc


_NC_CACHE = {}


def _get_nc():
    if "nc" not in _NC_CACHE:
        _NC_CACHE["nc"] = build_nc()
    return _NC_CACHE["nc"]


def _rotation_matrix_T():
    # rot(x)[2i] = -x[2i+1]; rot(x)[2i+1] = x[2i].  R[i,j] coefficient of x[j].
    R = np.zeros((HD, HD), np.float32)
    idx = np.arange(0, HD, 2)
    R[idx, idx + 1] = -1.0
    R[idx + 1, idx] = 1.0
    return np.ascontiguousarray(R.T)


def prepare_in_maps(hidden_states, sin, cos, Wq, Wk, Wv, Wo):
    hidden_states = np.asarray(hidden_states, dtype=np.float32)
    sin = np.asarray(sin, dtype=np.float32)
    cos = np.asarray(cos, dtype=np.float32)
    Wq = np.asarray(Wq, dtype=np.float32)
    Wk = np.asarray(Wk, dtype=np.float32)
    Wv = np.asarray(Wv, dtype=np.float32)
    Wo = np.asarray(Wo, dtype=np.float32)

    hT = np.ascontiguousarray(hidden_states[0].T).astype(NPBF16)
    ct = np.ascontiguousarray(np.repeat(cos, 2, axis=1).T).astype(NPBF16)
    st = np.ascontiguousarray(np.repeat(sin, 2, axis=1).T).astype(NPBF16)
    rot = _rotation_matrix_T().astype(NPBF16)

    in_maps = []
    for c in range(N_CORES):
        e0 = c * DC
        in_maps.append(
            {
                "hT": hT,
                "wq": np.ascontiguousarray(Wq[e0:e0 + DC, :].T).astype(NPBF16),
                "wk": np.ascontiguousarray(Wk[e0:e0 + DC, :].T).astype(NPBF16),
                "wv": np.ascontiguousarray(Wv[e0:e0 + DC, :].T).astype(NPBF16),
                "wo": np.ascontiguousarray(Wo[:, e0:e0 + DC].T).astype(NPBF16),
                "ct": ct,
                "st": st,
                "rot": rot,
            }
        )
    return in_maps


def kernel(hidden_states, attention_mask, sin, cos, Wq, Wk, Wv, Wo):
    in_maps = prepare_in_maps(hidden_states, sin, cos, Wq, Wk, Wv, Wo)
    nc = _get_nc()
    res = run_bass_kernel_spmd(nc, in_maps, list(range(N_CORES)))
    out = res.results[0]["out"].astype(np.float32)
    for c in range(1, N_CORES):
        out += res.results[c]["out"].astype(np.float32)
    return out[None]


# revision 31
# speedup vs baseline: 1.0181x; 1.0181x over previous
"""GPT-J joint attention (B=1, S=2048, D=2048, H=16, HD=128) on 8 Trainium2
NeuronCores, tensor-parallel over heads (2 heads per core).

Per-core program (all matmuls bf16 inputs, fp32 PSUM accumulation):
  - QT/KT = W[qk]_shard @ hidden^T        ([hd, s] layout, per head)
  - RoPE applied via a rotation-matrix matmul + elementwise combine
  - V = hidden @ Wv_shard^T               ([s, hd] layout)
  - scores^T tiles = KT_tile^T . QT_block ([k, q] layout) -> exp -> causal mask
  - O^T accumulated as V_tile^T . P^T; softmax denominator via ones-matmul
  - partial out = O^T{normalized}^T . Wo_shard^T, streamed to DRAM per row-block

Host side: shard/transpose/cast inputs, run SPMD on 8 cores, sum the 8
partial outputs (the tensor-parallel all-reduce equivalent).
"""
import sys

import numpy as np
import ml_dtypes

try:
    import concourse.bass as bass
except ImportError:  # pragma: no cover
    sys.path.insert(0, "/opt/trn_rl_repo")
    import concourse.bass as bass

import concourse.mybir as mybir
import concourse.tile as tile
from concourse.bass_utils import run_bass_kernel_spmd

BF16 = mybir.dt.bfloat16
F32 = mybir.dt.float32
NPBF16 = ml_dtypes.bfloat16

N_CORES = 8
S = 2048          # sequence length
D = 2048          # model dim
HD = 128          # head dim
NHC = 2           # heads per core
DC = NHC * HD     # shard width (256)
P = 128           # partitions
KD = D // P       # 16 contraction tiles over model dim
QBS = 512         # q-block size
NQB = S // QBS    # 4 q-blocks
NST = S // P      # 16 sequence tiles of 128
SCALE = 1.0 / float(np.sqrt(HD))

# ---------------------------------------------------------------------------
# Walrus's CoreV3 drain encoding accepts a single sem wait; Tile's tail drain
# carries one wait per logical proc. Split it into one drain per proc.
# ---------------------------------------------------------------------------


def _install_drain_split():
    if getattr(tile.TileContext, "_drain_split_installed", False):
        return
    from concourse.vector_clock import ScopedClock, VectorClock

    def _drain_and_barrier(self, tick_clock, wait_clock):
        full = tick_clock.global_clock
        n = len(full)
        for i in range(n):
            if full[i] <= 0:
                continue
            vec = [full[j] if j == i else 0 for j in range(n)]
            drain_inst = self.nc.sync.drain()
            wait_clock.add_sem_waits(
                drain_inst.ins, ScopedClock({None: VectorClock(vec)})
            )
        self.nc.all_engine_barrier()
        assert self.sems is not None
        popped = self.nc._tile_sem_poison_stack.pop()
        assert popped is self._sem_poison
        self.nc.clear_and_free_semaphores(list(self.sems.allocated().values()))
        self.nc.all_engine_barrier()

    tile.TileContext._drain_and_barrier = _drain_and_barrier
    tile.TileContext._drain_split_installed = True


def _split_excess_waits(nc, limit=1):
    """This walrus build rejects instructions carrying more than one sem wait
    (CoreV3 setupSyncWait: 'Too many sync wait commands'). Spill excess waits
    onto same-engine NOPs inserted just before the instruction — the engine
    executes them in queue order, so blocking semantics are unchanged."""
    ctr = 0
    for fn in nc.m.functions:
        for blk in fn.blocks:
            new_list = []
            for inst in blk.instructions:
                si = inst.sync_info
                if si is not None and len(si.on_wait) > limit:
                    waits = list(si.on_wait)
                    excess, keep = waits[:-limit], waits[-limit:]
                    for w in excess:
                        ctr += 1
                        nop = mybir.InstNoOp(
                            name=f"I-wsplit-{ctr}", text_hint="wait_split"
                        )
                        nop.engine = inst.engine
                        nop.sync_info = mybir.SyncInfo(on_wait=[w], on_update=[])
                        new_list.append(nop)
                    inst.sync_info = mybir.SyncInfo(
                        on_wait=keep, on_update=si.on_update
                    )
                new_list.append(inst)
            if len(new_list) != len(blk.instructions):
                blk.instructions[:] = new_list
    return ctr


def build_nc(split_waits=True):
    _install_drain_split()
    nc = bass.Bass()

    hT = nc.dram_tensor("hT", [D, S], BF16, kind="ExternalInput")
    wq = nc.dram_tensor("wq", [D, DC], BF16, kind="ExternalInput")
    wk = nc.dram_tensor("wk", [D, DC], BF16, kind="ExternalInput")
    wv = nc.dram_tensor("wv", [D, DC], BF16, kind="ExternalInput")
    wo = nc.dram_tensor("wo", [DC, D], BF16, kind="ExternalInput")
    ct = nc.dram_tensor("ct", [P, S], BF16, kind="ExternalInput")
    st = nc.dram_tensor("st", [P, S], BF16, kind="ExternalInput")
    rot = nc.dram_tensor("rot", [P, P], BF16, kind="ExternalInput")
    out = nc.dram_tensor("out", [S, D], BF16, kind="ExternalOutput")

    Exp = mybir.ActivationFunctionType.Exp
    Copy = mybir.ActivationFunctionType.Copy

    with tile.TileContext(nc) as tc:
        with (
            tc.tile_pool(name="const", bufs=1) as const,
            tc.tile_pool(name="acts", bufs=1) as acts,
            tc.tile_pool(name="work", bufs=2) as work,
            tc.tile_pool(name="ptpool", bufs=4) as ptpool,
            tc.tile_pool(name="outstage", bufs=2) as outstage,
            tc.tile_pool(name="ps_main", bufs=3, space="PSUM") as ps_main,
            tc.tile_pool(name="ps_op", bufs=2, space="PSUM") as ps_op,
            tc.tile_pool(name="ps_acc", bufs=2, space="PSUM") as ps_acc,
            tc.tile_pool(name="ps_misc", bufs=1, space="PSUM") as ps_misc,
        ):
            # ---- constants / weights into SBUF (few big DMAs: the HWDGE
            # trigger sequencer costs ~0.4us per dma_start, so batch) ----
            wq_sb = const.tile([P, KD, DC], BF16)
            wk_sb = const.tile([P, KD, DC], BF16)
            wv_sb = const.tile([P, KD, DC], BF16)
            hT_sb = const.tile([P, KD, S], BF16)
            hT_r = hT.rearrange("(kd p) s -> p kd s", p=P)
            ct_sb = const.tile([P, S], BF16)
            st_sb = const.tile([P, S], BF16)
            rot_sb = const.tile([P, P], BF16)
            wo_sb = const.tile([P, NHC, D], BF16)
            # order = first-use order: Wq, then the first q-block of hidden
            # (in 4 kd-chunks so matmuls can start on partial data), then the
            # other weights / rope constants, then the rest of hidden.
            nc.sync.dma_start(
                out=wq_sb, in_=wq.rearrange("(kd p) e -> p kd e", p=P)
            )
            for c4 in range(4):
                nc.sync.dma_start(
                    out=hT_sb[:, c4 * 4:(c4 + 1) * 4, 0:QBS],
                    in_=hT_r[:, c4 * 4:(c4 + 1) * 4, 0:QBS],
                )
            nc.sync.dma_start(
                out=wk_sb, in_=wk.rearrange("(kd p) e -> p kd e", p=P)
            )
            nc.sync.dma_start(out=rot_sb, in_=rot[:, :])
            nc.sync.dma_start(out=ct_sb, in_=ct[:, :])
            nc.sync.dma_start(out=st_sb, in_=st[:, :])
            nc.sync.dma_start(
                out=wv_sb, in_=wv.rearrange("(kd p) e -> p kd e", p=P)
            )
            for qb in range(1, NQB):
                nc.sync.dma_start(
                    out=hT_sb[:, :, qb * QBS:(qb + 1) * QBS],
                    in_=hT_r[:, :, qb * QBS:(qb + 1) * QBS],
                )
            nc.sync.dma_start(
                out=wo_sb, in_=wo.rearrange("(k2 p) e -> p k2 e", p=P)
            )
            ones_colb = const.tile([P, 1], BF16)   # lhsT for k-partition sums
            nc.vector.memset(ones_colb, 1.0)
            ones_row = const.tile([1, P], F32)     # lhsT for partition broadcast
            nc.vector.memset(ones_row, 1.0)

            # persistent activations
            qt_sb = acts.tile([P, NHC, S], BF16)   # [hd, h, s] rotary-applied Q^T
            kt_sb = acts.tile([P, NHC, S], BF16)
            v_sb = acts.tile([P, NST, DC], BF16)   # [s%128, s//128, head*hd]
            otb_sb = acts.tile([P, NHC, S], BF16)  # normalized O^T per head

            # ---- projections, q-block at a time (follows hT DMA order) ----
            # The rope epilogue (psum->sbuf copy, rotation matmul, 3 DVE ops)
            # of each 16-matmul block is deferred until after the NEXT block's
            # matmuls are emitted, so the PE never waits on it.
            pending = []  # (psum, dst_sb, h, qb)

            def flush_rope(keep=0):
                while len(pending) > keep:
                    ps, dst_sb, h, qb = pending.pop(0)
                    sl = slice(qb * QBS, (qb + 1) * QBS)
                    raw = work.tile([P, QBS], BF16, tag="raw")
                    nc.scalar.activation(raw, ps, Copy)
                    rps = ps_main.tile([P, QBS], F32, tag="mm")
                    nc.tensor.matmul(rps, lhsT=rot_sb, rhs=raw, start=True, stop=True)
                    t1 = work.tile([P, QBS], BF16, tag="t1")
                    t2 = work.tile([P, QBS], BF16, tag="t2")
                    nc.vector.tensor_mul(t1, raw, ct_sb[:, sl])
                    nc.vector.tensor_mul(t2, rps, st_sb[:, sl])
                    nc.vector.tensor_add(dst_sb[:, h, sl], t1, t2)

            def project(w_sb, dst_sb, h, qb):
                flush_rope(keep=1)
                sl = slice(qb * QBS, (qb + 1) * QBS)
                ps = ps_main.tile([P, QBS], F32, name="proj_ps", tag="mm")
                for kd in range(KD):
                    nc.tensor.matmul(
                        ps,
                        lhsT=w_sb[:, kd, h * HD:(h + 1) * HD],
                        rhs=hT_sb[:, kd, sl],
                        start=(kd == 0),
                        stop=(kd == KD - 1),
                    )
                pending.append((ps, dst_sb, h, qb))

            for qb in range(NQB):
                for h in range(NHC):
                    project(wq_sb, qt_sb, h, qb)
                    project(wk_sb, kt_sb, h, qb)
                for s4 in range(4):
                    flush_rope(keep=1)
                    st_idx = qb * 4 + s4
                    ps = ps_main.tile([P, DC], F32, tag="mm")
                    for kd in range(KD):
                        nc.tensor.matmul(
                            ps,
                            lhsT=hT_sb[:, kd, st_idx * P:(st_idx + 1) * P],
                            rhs=wv_sb[:, kd, :],
                            start=(kd == 0),
                            stop=(kd == KD - 1),
                        )
                    nc.scalar.activation(v_sb[:, st_idx, :], ps, Copy)
            flush_rope()

            # ---- attention + out-projection, pipelined per q-block ----
            # Both heads interleave inside one kt loop (fills PE bubbles while
            # exp/mask run), the P.V matmul lags the score matmul by one kt,
            # and out_proj for block qb-1 runs during qb's denominator chain.
            def make_outproj_chunks(qb):
                """One chunk = one [128,512] out tile (2 matmuls + a copy,
                plus the row-block DMA on the last chunk). Chunks are fed one
                at a time into the next q-block's attention loop so the PE
                fills the bubbles left by the exp-paced score pipeline."""
                state = {}
                chunks = []
                for s4 in range(4):
                    st_idx = qb * 4 + s4
                    for eb in range(NQB):
                        def chunk(s4=s4, st_idx=st_idx, eb=eb):
                            if eb == 0:
                                state[s4] = outstage.tile(
                                    [P, D], BF16, name="ost", tag="ost"
                                )
                            ost = state[s4]
                            ops = ps_op.tile(
                                [P, QBS], F32, name="ops", tag="op"
                            )
                            for h in range(NHC):
                                nc.tensor.matmul(
                                    ops,
                                    lhsT=otb_sb[:, h, st_idx * P:(st_idx + 1) * P],
                                    rhs=wo_sb[:, h, eb * QBS:(eb + 1) * QBS],
                                    start=(h == 0),
                                    stop=(h == NHC - 1),
                                )
                            nc.vector.tensor_copy(
                                ost[:, eb * QBS:(eb + 1) * QBS], ops
                            )
                            if eb == NQB - 1:
                                nc.sync.dma_start(
                                    out=out[st_idx * P:(st_idx + 1) * P, :],
                                    in_=ost,
                                )
                        chunks.append(chunk)
                return chunks

            OT_LAG = 2  # P.V matmul trails the score matmul by 2 kt steps so
            # its sem wait is already satisfied and LDWEIGHTS pipelines.
            pending_chunks = []
            for qb in range(NQB):
                qsl = slice(qb * QBS, (qb + 1) * QBS)
                kmax = (qb + 1) * 4
                ot_pss, pts = [], {}
                # 4-lane bf16 partial sums of exp tiles (softmax denominator);
                # all adds run in the DVE 16-bit fast mode, chains stay short.
                accs = [[None] * 4 for _ in range(NHC)]

                def acc_pt(h, kt, pt):
                    lane = kt % 4
                    if accs[h][lane] is None:
                        acc = work.tile(
                            [P, QBS], BF16, name=f"za{h}_{lane}",
                            tag=f"za{h}_{lane}",
                        )
                        nc.vector.tensor_copy(acc, pt)
                        accs[h][lane] = acc
                    else:
                        acc = accs[h][lane]
                        nc.vector.tensor_add(acc, acc, pt)

                def acc_merge(h):
                    lanes = [a for a in accs[h] if a is not None]
                    while len(lanes) > 1:
                        nxt = []
                        for i in range(0, len(lanes) - 1, 2):
                            nc.vector.tensor_add(lanes[i], lanes[i], lanes[i + 1])
                            nxt.append(lanes[i])
                        if len(lanes) % 2:
                            nxt.append(lanes[-1])
                        lanes = nxt
                    return lanes[0]

                for h in range(NHC):
                    ot_pss.append(ps_acc.tile([P, QBS], F32, name="ot_ps", tag="ps_ot"))

                def pv_step(kt):
                    for h in range(NHC):
                        nc.tensor.matmul(
                            ot_pss[h],
                            lhsT=v_sb[:, kt, h * HD:(h + 1) * HD],
                            rhs=pts[(h, kt)],
                            start=(kt == 0),
                            stop=(kt == kmax - 1),
                            skip_group_check=True,
                        )

                for kt in range(kmax):
                    for h in range(NHC):
                        sps = ps_main.tile([P, QBS], F32, tag="mm")
                        nc.tensor.matmul(
                            sps,
                            lhsT=kt_sb[:, h, kt * P:(kt + 1) * P],
                            rhs=qt_sb[:, h, qsl],
                            start=True,
                            stop=True,
                        )
                        pt = ptpool.tile([P, QBS], BF16, tag=f"pt{h}")
                        j = kt - qb * 4
                        if j < 0:
                            nc.scalar.activation(pt, sps, Exp, scale=SCALE)
                        else:
                            # diagonal tile: columns < 128j are fully masked,
                            # the causal boundary runs through the next 128.
                            c0 = P * j
                            if c0 > 0:
                                nc.gpsimd.memset(pt[:, 0:c0], 0.0)
                            nc.scalar.activation(
                                pt[:, c0:], sps[:, c0:], Exp, scale=SCALE
                            )
                            nc.gpsimd.affine_select(
                                out=pt[:, c0:c0 + P],
                                in_=pt[:, c0:c0 + P],
                                compare_op=mybir.AluOpType.is_ge,
                                fill=0.0,
                                base=qb * QBS - kt * P + c0,
                                pattern=[[1, P]],
                                channel_multiplier=-1,
                            )
                        pts[(h, kt)] = pt
                        acc_pt(h, kt, pt)
                        if pending_chunks:
                            pending_chunks.pop(0)()
                    if kt >= OT_LAG:
                        pv_step(kt - OT_LAG)
                for kt in range(max(kmax - OT_LAG, 0), kmax):
                    pv_step(kt)

                den_sbs = []
                for h in range(NHC):
                    lanes = [a for a in accs[h] if a is not None]
                    den_ps = ps_misc.tile([1, QBS], F32, name="den_ps", tag="ps_dm")
                    for li, lane in enumerate(lanes):
                        nc.tensor.matmul(
                            den_ps, lhsT=ones_colb, rhs=lane,
                            start=(li == 0), stop=(li == len(lanes) - 1),
                            skip_group_check=True,
                        )
                    r_sb = work.tile([1, QBS], F32, tag=f"r{h}")
                    nc.vector.reciprocal(r_sb, den_ps)
                    den_sbs.append(r_sb)
                while pending_chunks:
                    pending_chunks.pop(0)()
                for h in range(NHC):
                    bc_ps = ps_misc.tile([P, QBS], F32, name="bc_ps", tag="ps_dm")
                    nc.tensor.matmul(
                        bc_ps, lhsT=ones_row, rhs=den_sbs[h], start=True, stop=True
                    )
                    bc_sb = work.tile([P, QBS], F32, tag=f"bc{h}")
                    nc.vector.tensor_copy(bc_sb, bc_ps)
                    nc.vector.tensor_mul(otb_sb[:, h, qsl], ot_pss[h], bc_sb)
                pending_chunks = make_outproj_chunks(qb)

            while pending_chunks:
                pending_chunks.pop(0)()
    if split_waits:
        _split_excess_waits(nc)
    return nc


_NC_CACHE = {}


def _get_nc():
    if "nc" not in _NC_CACHE:
        _NC_CACHE["nc"] = build_nc()
    return _NC_CACHE["nc"]


def _rotation_matrix_T():
    # rot(x)[2i] = -x[2i+1]; rot(x)[2i+1] = x[2i].  R[i,j] coefficient of x[j].
    R = np.zeros((HD, HD), np.float32)
    idx = np.arange(0, HD, 2)
    R[idx, idx + 1] = -1.0
    R[idx + 1, idx] = 1.0
    return np.ascontiguousarray(R.T)


def prepare_in_maps(hidden_states, sin, cos, Wq, Wk, Wv, Wo):
    hidden_states = np.asarray(hidden_states, dtype=np.float32)
    sin = np.asarray(sin, dtype=np.float32)
    cos = np.asarray(cos, dtype=np.float32)
    Wq = np.asarray(Wq, dtype=np.float32)
    Wk = np.asarray(Wk, dtype=np.float32)
    Wv = np.asarray(Wv, dtype=np.float32)
    Wo = np.asarray(Wo, dtype=np.float32)

    hT = np.ascontiguousarray(hidden_states[0].T).astype(NPBF16)
    ct = np.ascontiguousarray(np.repeat(cos, 2, axis=1).T).astype(NPBF16)
    st = np.ascontiguousarray(np.repeat(sin, 2, axis=1).T).astype(NPBF16)
    rot = _rotation_matrix_T().astype(NPBF16)

    in_maps = []
    for c in range(N_CORES):
        e0 = c * DC
        in_maps.append(
            {
                "hT": hT,
                "wq": np.ascontiguousarray(Wq[e0:e0 + DC, :].T).astype(NPBF16),
                "wk": np.ascontiguousarray(Wk[e0:e0 + DC, :].T).astype(NPBF16),
                "wv": np.ascontiguousarray(Wv[e0:e0 + DC, :].T).astype(NPBF16),
                "wo": np.ascontiguousarray(Wo[:, e0:e0 + DC].T).astype(NPBF16),
                "ct": ct,
                "st": st,
                "rot": rot,
            }
        )
    return in_maps


def kernel(hidden_states, attention_mask, sin, cos, Wq, Wk, Wv, Wo):
    in_maps = prepare_in_maps(hidden_states, sin, cos, Wq, Wk, Wv, Wo)
    nc = _get_nc()
    res = run_bass_kernel_spmd(nc, in_maps, list(range(N_CORES)))
    out = res.results[0]["out"].astype(np.float32)
    for c in range(1, N_CORES):
        out += res.results[c]["out"].astype(np.float32)
    return out[None]


# revision 37
# speedup vs baseline: 1.0982x; 1.0786x over previous
"""GPT-J joint attention (B=1, S=2048, D=2048, H=16, HD=128) on 8 Trainium2
NeuronCores, tensor-parallel over heads (2 heads per core).

Per-core program (all matmuls bf16 inputs, fp32 PSUM accumulation):
  - QT/KT = W[qk]_shard @ hidden^T        ([hd, s] layout, per head)
  - RoPE applied via a rotation-matrix matmul + elementwise combine
  - V = hidden @ Wv_shard^T               ([s, hd] layout)
  - scores^T tiles = KT_tile^T . QT_block ([k, q] layout) -> exp -> causal mask
  - O^T accumulated as V_tile^T . P^T; softmax denominator via ones-matmul
  - partial out = O^T{normalized}^T . Wo_shard^T, streamed to DRAM per row-block

Host side: shard/transpose/cast inputs, run SPMD on 8 cores, sum the 8
partial outputs (the tensor-parallel all-reduce equivalent).
"""
import sys

import numpy as np
import ml_dtypes

try:
    import concourse.bass as bass
except ImportError:  # pragma: no cover
    sys.path.insert(0, "/opt/trn_rl_repo")
    import concourse.bass as bass

import concourse.mybir as mybir
import concourse.tile as tile
from concourse.bass_utils import run_bass_kernel_spmd

BF16 = mybir.dt.bfloat16
F32 = mybir.dt.float32
NPBF16 = ml_dtypes.bfloat16

N_CORES = 8
S = 2048          # sequence length
D = 2048          # model dim
HD = 128          # head dim
NHC = 2           # heads per core
DC = NHC * HD     # shard width (256)
P = 128           # partitions
KD = D // P       # 16 contraction tiles over model dim
QBS = 512         # q-block size
NQB = S // QBS    # 4 q-blocks
NST = S // P      # 16 sequence tiles of 128
SCALE = 1.0 / float(np.sqrt(HD))

# ---------------------------------------------------------------------------
# Walrus's CoreV3 drain encoding accepts a single sem wait; Tile's tail drain
# carries one wait per logical proc. Split it into one drain per proc.
# ---------------------------------------------------------------------------


def _install_drain_split():
    if getattr(tile.TileContext, "_drain_split_installed", False):
        return
    from concourse.vector_clock import ScopedClock, VectorClock

    def _drain_and_barrier(self, tick_clock, wait_clock):
        full = tick_clock.global_clock
        n = len(full)
        for i in range(n):
            if full[i] <= 0:
                continue
            vec = [full[j] if j == i else 0 for j in range(n)]
            drain_inst = self.nc.sync.drain()
            wait_clock.add_sem_waits(
                drain_inst.ins, ScopedClock({None: VectorClock(vec)})
            )
        self.nc.all_engine_barrier()
        assert self.sems is not None
        popped = self.nc._tile_sem_poison_stack.pop()
        assert popped is self._sem_poison
        self.nc.clear_and_free_semaphores(list(self.sems.allocated().values()))
        self.nc.all_engine_barrier()

    tile.TileContext._drain_and_barrier = _drain_and_barrier
    tile.TileContext._drain_split_installed = True


def _split_excess_waits(nc, limit=1):
    """This walrus build rejects instructions carrying more than one sem wait
    (CoreV3 setupSyncWait: 'Too many sync wait commands'). Spill excess waits
    onto same-engine NOPs inserted just before the instruction — the engine
    executes them in queue order, so blocking semantics are unchanged."""
    ctr = 0
    for fn in nc.m.functions:
        for blk in fn.blocks:
            new_list = []
            for inst in blk.instructions:
                si = inst.sync_info
                if si is not None and len(si.on_wait) > limit:
                    waits = list(si.on_wait)
                    excess, keep = waits[:-limit], waits[-limit:]
                    for w in excess:
                        ctr += 1
                        nop = mybir.InstNoOp(
                            name=f"I-wsplit-{ctr}", text_hint="wait_split"
                        )
                        nop.engine = inst.engine
                        nop.sync_info = mybir.SyncInfo(on_wait=[w], on_update=[])
                        new_list.append(nop)
                    inst.sync_info = mybir.SyncInfo(
                        on_wait=keep, on_update=si.on_update
                    )
                new_list.append(inst)
            if len(new_list) != len(blk.instructions):
                blk.instructions[:] = new_list
    return ctr


def build_nc(split_waits=True):
    _install_drain_split()
    nc = bass.Bass()

    hT = nc.dram_tensor("hT", [D, S], BF16, kind="ExternalInput")
    wq = nc.dram_tensor("wq", [D, DC], BF16, kind="ExternalInput")
    wk = nc.dram_tensor("wk", [D, DC], BF16, kind="ExternalInput")
    wv = nc.dram_tensor("wv", [D, DC], BF16, kind="ExternalInput")
    wo = nc.dram_tensor("wo", [DC, D], BF16, kind="ExternalInput")
    ct = nc.dram_tensor("ct", [P, S], BF16, kind="ExternalInput")
    st = nc.dram_tensor("st", [P, S], BF16, kind="ExternalInput")
    rot = nc.dram_tensor("rot", [P, P], BF16, kind="ExternalInput")
    out = nc.dram_tensor("out", [S, D], BF16, kind="ExternalOutput")

    Exp = mybir.ActivationFunctionType.Exp
    Copy = mybir.ActivationFunctionType.Copy

    with tile.TileContext(nc) as tc:
        with (
            tc.tile_pool(name="const", bufs=1) as const,
            tc.tile_pool(name="acts", bufs=1) as acts,
            tc.tile_pool(name="work", bufs=2) as work,
            tc.tile_pool(name="ptpool", bufs=5) as ptpool,
            tc.tile_pool(name="outstage", bufs=3) as outstage,
            tc.tile_pool(name="ps_main", bufs=3, space="PSUM") as ps_main,
            tc.tile_pool(name="ps_op", bufs=2, space="PSUM") as ps_op,
            tc.tile_pool(name="ps_acc", bufs=2, space="PSUM") as ps_acc,
            tc.tile_pool(name="ps_misc", bufs=1, space="PSUM") as ps_misc,
        ):
            # ---- constants / weights into SBUF (few big DMAs: the HWDGE
            # trigger sequencer costs ~0.4us per dma_start, so batch) ----
            wq_sb = const.tile([P, KD, DC], BF16)
            wk_sb = const.tile([P, KD, DC], BF16)
            wv_sb = const.tile([P, KD, DC], BF16)
            hT_sb = const.tile([P, KD, S], BF16)
            hT_r = hT.rearrange("(kd p) s -> p kd s", p=P)
            ct_sb = const.tile([P, S], BF16)
            st_sb = const.tile([P, S], BF16)
            rot_sb = const.tile([P, P], BF16)
            wo_sb = const.tile([P, NHC, D], BF16)
            # order = first-use order: Wq, then the first q-block of hidden
            # (in 4 kd-chunks so matmuls can start on partial data), then the
            # other weights / rope constants, then the rest of hidden.
            nc.sync.dma_start(
                out=wq_sb, in_=wq.rearrange("(kd p) e -> p kd e", p=P)
            )
            for c4 in range(4):
                nc.sync.dma_start(
                    out=hT_sb[:, c4 * 4:(c4 + 1) * 4, 0:QBS],
                    in_=hT_r[:, c4 * 4:(c4 + 1) * 4, 0:QBS],
                )
            nc.sync.dma_start(
                out=wk_sb, in_=wk.rearrange("(kd p) e -> p kd e", p=P)
            )
            nc.sync.dma_start(out=rot_sb, in_=rot[:, :])
            nc.sync.dma_start(out=ct_sb, in_=ct[:, :])
            nc.sync.dma_start(out=st_sb, in_=st[:, :])
            nc.sync.dma_start(
                out=wv_sb, in_=wv.rearrange("(kd p) e -> p kd e", p=P)
            )
            for qb in range(1, NQB):
                nc.sync.dma_start(
                    out=hT_sb[:, :, qb * QBS:(qb + 1) * QBS],
                    in_=hT_r[:, :, qb * QBS:(qb + 1) * QBS],
                )
            nc.sync.dma_start(
                out=wo_sb, in_=wo.rearrange("(k2 p) e -> p k2 e", p=P)
            )
            ones_colb = const.tile([P, 1], BF16)   # lhsT for k-partition sums
            nc.vector.memset(ones_colb, 1.0)
            ones_row = const.tile([1, P], F32)     # lhsT for partition broadcast
            nc.vector.memset(ones_row, 1.0)

            # persistent activations
            qt_sb = acts.tile([P, NHC, S], BF16)   # [hd, h, s] rotary-applied Q^T
            kt_sb = acts.tile([P, NHC, S], BF16)
            v_sb = acts.tile([P, NST, DC], BF16)   # [s%128, s//128, head*hd]
            otb_sb = acts.tile([P, NHC, S], BF16)  # normalized O^T per head

            # ---- projections, q-block at a time (follows hT DMA order) ----
            # The rope epilogue (psum->sbuf copy, rotation matmul, 3 DVE ops)
            # of each 16-matmul block is deferred until after the NEXT block's
            # matmuls are emitted, so the PE never waits on it.
            pending = []  # (psum, dst_sb, h, qb)

            def flush_rope(keep=0):
                while len(pending) > keep:
                    ps, dst_sb, h, qb = pending.pop(0)
                    sl = slice(qb * QBS, (qb + 1) * QBS)
                    raw = work.tile([P, QBS], BF16, tag="raw")
                    nc.scalar.activation(raw, ps, Copy)
                    rps = ps_main.tile([P, QBS], F32, tag="mm")
                    nc.tensor.matmul(rps, lhsT=rot_sb, rhs=raw, start=True, stop=True)
                    t1 = work.tile([P, QBS], BF16, tag="t1")
                    t2 = work.tile([P, QBS], BF16, tag="t2")
                    nc.vector.tensor_mul(t1, raw, ct_sb[:, sl])
                    nc.vector.tensor_mul(t2, rps, st_sb[:, sl])
                    nc.vector.tensor_add(dst_sb[:, h, sl], t1, t2)

            def project(w_sb, dst_sb, h, qb):
                flush_rope(keep=1)
                sl = slice(qb * QBS, (qb + 1) * QBS)
                ps = ps_main.tile([P, QBS], F32, name="proj_ps", tag="mm")
                for kd in range(KD):
                    nc.tensor.matmul(
                        ps,
                        lhsT=w_sb[:, kd, h * HD:(h + 1) * HD],
                        rhs=hT_sb[:, kd, sl],
                        start=(kd == 0),
                        stop=(kd == KD - 1),
                    )
                pending.append((ps, dst_sb, h, qb))

            for qb in range(NQB):
                for h in range(NHC):
                    project(wq_sb, qt_sb, h, qb)
                    project(wk_sb, kt_sb, h, qb)
                for s4 in range(4):
                    flush_rope(keep=1)
                    st_idx = qb * 4 + s4
                    ps = ps_main.tile([P, DC], F32, tag="mm")
                    for kd in range(KD):
                        nc.tensor.matmul(
                            ps,
                            lhsT=hT_sb[:, kd, st_idx * P:(st_idx + 1) * P],
                            rhs=wv_sb[:, kd, :],
                            start=(kd == 0),
                            stop=(kd == KD - 1),
                        )
                    nc.scalar.activation(v_sb[:, st_idx, :], ps, Copy)
            flush_rope()

            # ---- attention + out-projection, pipelined per q-block ----
            # Both heads interleave inside one kt loop (fills PE bubbles while
            # exp/mask run), the P.V matmul lags the score matmul by one kt,
            # and out_proj for block qb-1 runs during qb's denominator chain.
            def make_outproj_chunks(qb):
                """One chunk = one [128,512] out tile (2 matmuls + a copy,
                plus the row-block DMA on the last chunk). Chunks are fed one
                at a time into the next q-block's attention loop so the PE
                fills the bubbles left by the exp-paced score pipeline."""
                state = {}
                chunks = []
                for s4 in range(4):
                    st_idx = qb * 4 + s4
                    for eb in range(NQB):
                        def chunk(s4=s4, st_idx=st_idx, eb=eb):
                            if eb == 0:
                                state[s4] = outstage.tile(
                                    [P, D], BF16, name="ost", tag="ost"
                                )
                            ost = state[s4]
                            ops = ps_op.tile(
                                [P, QBS], F32, name="ops", tag="op"
                            )
                            for h in range(NHC):
                                nc.tensor.matmul(
                                    ops,
                                    lhsT=otb_sb[:, h, st_idx * P:(st_idx + 1) * P],
                                    rhs=wo_sb[:, h, eb * QBS:(eb + 1) * QBS],
                                    start=(h == 0),
                                    stop=(h == NHC - 1),
                                )
                            nc.scalar.activation(
                                ost[:, eb * QBS:(eb + 1) * QBS], ops, Copy
                            )
                            if eb == NQB - 1:
                                nc.sync.dma_start(
                                    out=out[st_idx * P:(st_idx + 1) * P, :],
                                    in_=ost,
                                )
                        chunks.append(chunk)
                return chunks

            OT_LAG = 3  # P.V matmul trails the score matmul by 2 kt steps so
            # its sem wait is already satisfied and LDWEIGHTS pipelines.
            pending_chunks = []
            for qb in range(NQB):
                qsl = slice(qb * QBS, (qb + 1) * QBS)
                kmax = (qb + 1) * 4
                ot_pss, pts = [], {}
                # 4-lane bf16 partial sums of exp tiles (softmax denominator);
                # all adds run in the DVE 16-bit fast mode, chains stay short.
                accs = [[None] * 4 for _ in range(NHC)]

                def acc_pt(h, kt, pt):
                    lane = kt % 4
                    if accs[h][lane] is None:
                        acc = work.tile(
                            [P, QBS], BF16, name=f"za{h}_{lane}",
                            tag=f"za{h}_{lane}",
                        )
                        nc.vector.tensor_copy(acc, pt)
                        accs[h][lane] = acc
                    else:
                        acc = accs[h][lane]
                        nc.vector.tensor_add(acc, acc, pt)

                def acc_merge(h):
                    lanes = [a for a in accs[h] if a is not None]
                    while len(lanes) > 1:
                        nxt = []
                        for i in range(0, len(lanes) - 1, 2):
                            nc.vector.tensor_add(lanes[i], lanes[i], lanes[i + 1])
                            nxt.append(lanes[i])
                        if len(lanes) % 2:
                            nxt.append(lanes[-1])
                        lanes = nxt
                    return lanes[0]

                for h in range(NHC):
                    ot_pss.append(ps_acc.tile([P, QBS], F32, name="ot_ps", tag="ps_ot"))

                def pv_step(kt):
                    for h in range(NHC):
                        nc.tensor.matmul(
                            ot_pss[h],
                            lhsT=v_sb[:, kt, h * HD:(h + 1) * HD],
                            rhs=pts[(h, kt)],
                            start=(kt == kt_order[0]),
                            stop=(kt == kt_order[-1]),
                            skip_group_check=True,
                        )

                kt_order = list(range(qb * 4, kmax)) + list(range(0, qb * 4))
                for ki, kt in enumerate(kt_order):
                    for h in range(NHC):
                        sps = ps_main.tile([P, QBS], F32, tag="mm")
                        nc.tensor.matmul(
                            sps,
                            lhsT=kt_sb[:, h, kt * P:(kt + 1) * P],
                            rhs=qt_sb[:, h, qsl],
                            start=True,
                            stop=True,
                        )
                        pt = ptpool.tile([P, QBS], BF16, tag=f"pt{h}")
                        nc.scalar.activation(pt, sps, Exp, scale=SCALE)
                        j = kt - qb * 4
                        if j >= 0:  # diagonal tile: causal mask (on Pool)
                            w = min(P * (j + 1), QBS)
                            nc.gpsimd.affine_select(
                                out=pt[:, 0:w],
                                in_=pt[:, 0:w],
                                compare_op=mybir.AluOpType.is_ge,
                                fill=0.0,
                                base=qb * QBS - kt * P,
                                pattern=[[1, w]],
                                channel_multiplier=-1,
                            )
                        pts[(h, kt)] = pt
                        acc_pt(h, kt, pt)
                        if pending_chunks:
                            pending_chunks.pop(0)()
                    if kt >= OT_LAG:
                        pv_step(kt - OT_LAG)
                for kt in range(max(kmax - OT_LAG, 0), kmax):
                    pv_step(kt)

                den_sbs = []
                for h in range(NHC):
                    lanes = [a for a in accs[h] if a is not None]
                    den_ps = ps_misc.tile([1, QBS], F32, name="den_ps", tag="ps_dm")
                    for li, lane in enumerate(lanes):
                        nc.tensor.matmul(
                            den_ps, lhsT=ones_colb, rhs=lane,
                            start=(li == 0), stop=(li == len(lanes) - 1),
                            skip_group_check=True,
                        )
                    r_sb = work.tile([1, QBS], F32, tag=f"r{h}")
                    nc.vector.reciprocal(r_sb, den_ps)
                    den_sbs.append(r_sb)
                while pending_chunks:
                    pending_chunks.pop(0)()
                for h in range(NHC):
                    bc_ps = ps_misc.tile([P, QBS], F32, name="bc_ps", tag="ps_dm")
                    nc.tensor.matmul(
                        bc_ps, lhsT=ones_row, rhs=den_sbs[h], start=True, stop=True
                    )
                    bc_sb = work.tile([P, QBS], F32, tag=f"bc{h}")
                    nc.vector.tensor_copy(bc_sb, bc_ps)
                    nc.vector.tensor_mul(otb_sb[:, h, qsl], ot_pss[h], bc_sb)
                pending_chunks = make_outproj_chunks(qb)

            while pending_chunks:
                pending_chunks.pop(0)()
    if split_waits:
        _split_excess_waits(nc)
    return nc


_NC_CACHE = {}


def _get_nc():
    if "nc" not in _NC_CACHE:
        _NC_CACHE["nc"] = build_nc()
    return _NC_CACHE["nc"]


def _rotation_matrix_T():
    # rot(x)[2i] = -x[2i+1]; rot(x)[2i+1] = x[2i].  R[i,j] coefficient of x[j].
    R = np.zeros((HD, HD), np.float32)
    idx = np.arange(0, HD, 2)
    R[idx, idx + 1] = -1.0
    R[idx + 1, idx] = 1.0
    return np.ascontiguousarray(R.T)


def prepare_in_maps(hidden_states, sin, cos, Wq, Wk, Wv, Wo):
    hidden_states = np.asarray(hidden_states, dtype=np.float32)
    sin = np.asarray(sin, dtype=np.float32)
    cos = np.asarray(cos, dtype=np.float32)
    Wq = np.asarray(Wq, dtype=np.float32)
    Wk = np.asarray(Wk, dtype=np.float32)
    Wv = np.asarray(Wv, dtype=np.float32)
    Wo = np.asarray(Wo, dtype=np.float32)

    hT = np.ascontiguousarray(hidden_states[0].T).astype(NPBF16)
    ct = np.ascontiguousarray(np.repeat(cos, 2, axis=1).T).astype(NPBF16)
    st = np.ascontiguousarray(np.repeat(sin, 2, axis=1).T).astype(NPBF16)
    rot = _rotation_matrix_T().astype(NPBF16)

    in_maps = []
    for c in range(N_CORES):
        e0 = c * DC
        in_maps.append(
            {
                "hT": hT,
                "wq": np.ascontiguousarray(Wq[e0:e0 + DC, :].T).astype(NPBF16),
                "wk": np.ascontiguousarray(Wk[e0:e0 + DC, :].T).astype(NPBF16),
                "wv": np.ascontiguousarray(Wv[e0:e0 + DC, :].T).astype(NPBF16),
                "wo": np.ascontiguousarray(Wo[:, e0:e0 + DC].T).astype(NPBF16),
                "ct": ct,
                "st": st,
                "rot": rot,
            }
        )
    return in_maps


def kernel(hidden_states, attention_mask, sin, cos, Wq, Wk, Wv, Wo):
    in_maps = prepare_in_maps(hidden_states, sin, cos, Wq, Wk, Wv, Wo)
    nc = _get_nc()
    res = run_bass_kernel_spmd(nc, in_maps, list(range(N_CORES)))
    out = res.results[0]["out"].astype(np.float32)
    for c in range(1, N_CORES):
        out += res.results[c]["out"].astype(np.float32)
    return out[None]
